# revision 23
# baseline (speedup 1.0000x reference)
"""Trainium2 Bass kernel for nn_GRU_Classifier (2-layer GRU + classifier head).

Problem (hardcoded shapes):
  B=2048, IN=512, H=512, OUT=256, N=32768.
  x is replicated T times (T = max bincount of pre_set_idx) as the input
  sequence of a 2-layer GRU; output head projects the first counts[b]
  timesteps of each batch row, gathered into a flat [N, OUT] tensor.

Strategy:
  - Data-parallel: shard batch B across 8 NeuronCores (256 rows/core),
    GRU weights replicated. All ragged indexing is resolved host-side.
  - Feature-major on-chip layout (features on partitions, batch on free
    dim): hidden state h is [H=4x128, 256], so h is directly the moving
    operand of every matmul -- the kernel needs zero transposes.
  - Matmuls run as float32r (fp32 data, relaxed-precision PE mode, 1
    cycle/row at free-dim 256); gate math is fp32 on ACT (sigmoid/tanh,
    per-partition biases) and DVE (tensor-tensor ops).
  - Layer-0's input projection xp0 = Wih0 @ x^T + biases is computed once;
    each step it is injected into PSUM with an identity matmul so the
    Whh accumulation lands on top of it (keeps gate math 1 ACT op).
  - Per-step output head: o_t = Wout @ h1_t + bout -> DRAM [T, OUT, 256];
    the host gathers (b, t<counts[b]) columns into the final [N, OUT].
"""

import os
import sys

import numpy as np

if "/opt/trn_rl_repo" not in sys.path:
    sys.path.insert(0, "/opt/trn_rl_repo")

B, IN, H, OUT = 2048, 512, 512, 256
NCORES = 8
BC = B // NCORES  # 256 batch rows per core
KT = H // 128  # 4 contraction tiles
G3 = 3 * H  # 1536 gate rows

# input blob layout (free-dim offsets, fp32 elements per partition)
OFF_W0X = 0
OFF_W0H = OFF_W0X + 4 * G3
OFF_W1X = OFF_W0H + 4 * G3
OFF_W1H = OFF_W1X + 4 * G3
OFF_WO = OFF_W1H + 4 * G3
OFF_XT = OFF_WO + 4 * OUT
OFF_EYE = OFF_XT + 4 * BC
OFF_XB0 = OFF_EYE + 128
OFF_B1RZ = OFF_XB0 + 12
OFF_B0NH = OFF_B1RZ + 8
OFF_B1NH = OFF_B0NH + 4
OFF_B1NX = OFF_B1NH + 4
OFF_BO = OFF_B1NX + 4
OFF_ZERO = OFF_BO + 2  # 4*BC zeros: initial h0/h1 state
FTOT = OFF_ZERO + 4 * BC

_BUILD_CACHE = {}


def _build(T):
    """Build the Bass module for a T-step run. Returns nc."""
    import concourse.mybir as mybir
    import concourse.tile as tile
    from concourse import bacc

    f32 = mybir.dt.float32
    AF = mybir.ActivationFunctionType

    f32r = mybir.dt.float32r
    nc = bacc.Bacc(trn_type="TRN2", target_bir_lowering=False, debug=False)

    # ---- DRAM I/O ----
    # ALL inputs ship as ONE [128, FTOT] f32r blob loaded by ONE DMA: the
    # fused f32r matmuls have a single sync-wait slot, so every matmul
    # operand must trace back to at most one DMA semaphore. Biases are f32
    # bitcast views into the same blob.
    blob = nc.dram_tensor("blob", [128, FTOT], f32r, kind="ExternalInput").ap()
    out = nc.dram_tensor("out", [T, 2, 128, BC], f32, kind="ExternalOutput").ap()

    with TileCtx(nc, tile) as (tc, pools):
        wp, cp, sp, gp, pp, px = pools

        # ---- load everything with a single DMA ----
        BLOB = wp.tile([128, FTOT], f32r, tag="blob")
        nc.gpsimd.dma_start(out=BLOB, in_=blob)

        def seg(off, n):
            return BLOB[:, off:off + n]

        def wseg(off, m):  # (4, m) weight block view [128, kt, m]
            return seg(off, 4 * m).rearrange("p (k m) -> p k m", m=m)

        W0X = wseg(OFF_W0X, G3)
        W0H = wseg(OFF_W0H, G3)
        W1X = wseg(OFF_W1X, G3)
        W1H = wseg(OFF_W1H, G3)
        WO = wseg(OFF_WO, OUT)
        XTS = wseg(OFF_XT, BC)
        EYE = seg(OFF_EYE, 128)
        XB0 = seg(OFF_XB0, 12).bitcast(f32)
        B1RZ = seg(OFF_B1RZ, 8).bitcast(f32)
        B0NH = seg(OFF_B0NH, 4).bitcast(f32)
        B1NH = seg(OFF_B1NH, 4).bitcast(f32)
        B1NX = seg(OFF_B1NX, 4).bitcast(f32)
        BO = seg(OFF_BO, 2).bitcast(f32)

        def mm(ps, lhsT, rhs, start, stop):
            nc.tensor.matmul(ps, lhsT, rhs, start=start, stop=stop)

        # ---- xp0 = Wih0 @ x^T + (bih0 + bhh0 for r,z; bih0 for n) ----
        XP0 = cp.tile([128, 12, BC], f32r, tag="xp0")
        for m in range(12):
            ps = pp.tile([128, BC], f32, tag="g")
            for k in range(KT):
                mm(ps, W0X[:, k, 128 * m:128 * (m + 1)], XTS[:, k, :],
                   k == 0, k == KT - 1)
            nc.scalar.activation(XP0[:, m, :], ps, AF.Identity,
                                 bias=XB0[:, m:m + 1])

        # ---- state init: zeros region of the blob (no memset needed) ----
        zero = seg(OFF_ZERO, 4 * BC).rearrange("p (k b) -> p k b", b=BC)
        h0 = h1 = zero

        def flat(ap):  # [128, 4, BC] -> [128, 4*BC] for full-width elementwise
            return ap.rearrange("p k b -> p (k b)")

        def ps_g():
            # 1-bank psum subtile slots; 6 in flight + the 2-bank "x" slot = 8
            return pp.tile([128, BC], f32, tag="g", name="psg")

        def emit_l0_mm(h0p):
            """r,z: identity-inject XP0 then Whh0; n: Whh0 only. 56 MMs."""
            psr, psz, psn = [], [], []
            for g, lst in ((0, psr), (1, psz)):
                for mi in range(4):
                    m = 4 * g + mi
                    ps = ps_g()
                    mm(ps, EYE, XP0[:, m, :], True, False)
                    for k in range(KT):
                        mm(ps, W0H[:, k, 128 * m:128 * (m + 1)], h0p[:, k, :],
                           False, k == KT - 1)
                    lst.append(ps)
            for mi in range(4):
                m = 8 + mi
                ps = ps_g()
                for k in range(KT):
                    mm(ps, W0H[:, k, 128 * m:128 * (m + 1)], h0p[:, k, :],
                       k == 0, k == KT - 1)
                psn.append(ps)
            return psr, psz, psn

        def emit_l1_mm(h0c, h1p):
            """r,z: Wih1@h0 + Whh1@h1; n: separate h- and x-side psums."""
            psr, psz, psn = [], [], []
            for g, lst in ((0, psr), (1, psz)):
                for mi in range(4):
                    m = 4 * g + mi
                    ps = ps_g()
                    for k in range(KT):
                        mm(ps, W1X[:, k, 128 * m:128 * (m + 1)], h0c[:, k, :],
                           k == 0, False)
                    for k in range(KT):
                        mm(ps, W1H[:, k, 128 * m:128 * (m + 1)], h1p[:, k, :],
                           False, k == KT - 1)
                    lst.append(ps)
            for mi in range(4):
                m = 8 + mi
                ps = ps_g()
                for k in range(KT):
                    mm(ps, W1H[:, k, 128 * m:128 * (m + 1)], h1p[:, k, :],
                       k == 0, k == KT - 1)
                psn.append(ps)
            psx = px.tile([128, 4, BC], f32, tag="x")
            for mi in range(4):
                m = 8 + mi
                for k in range(KT):
                    mm(psx[:, mi, :], W1X[:, k, 128 * m:128 * (m + 1)],
                       h0c[:, k, :], k == 0, k == KT - 1)
            return psr, psz, psn, psx

        def emit_out_mm(h1t):
            pso = px.tile([128, 2, BC], f32, tag="x")
            for mi in range(2):
                for k in range(KT):
                    mm(pso[:, mi, :], WO[:, k, 128 * mi:128 * (mi + 1)],
                       h1t[:, k, :], k == 0, k == KT - 1)
            return pso

        def emit_out_copy(pso, t):
            osb = gp.tile([128, 2, BC], f32, tag="o", bufs=3)
            for mi in range(2):
                nc.scalar.activation(osb[:, mi, :], pso[:, mi, :], AF.Identity,
                                     bias=BO[:, mi:mi + 1])
            nc.sync.dma_start(out=out[t].rearrange("f p b -> p f b"), in_=osb)

        def emit_l0_gates(psr, psz, psn, h0p):
            r0 = gp.tile([128, KT, BC], f32, tag="r")
            z0 = gp.tile([128, KT, BC], f32, tag="z")
            t1 = gp.tile([128, KT, BC], f32, tag="t1")
            n0 = gp.tile([128, KT, BC], f32, tag="n")
            d0 = gp.tile([128, KT, BC], f32, tag="d")
            for mi in range(4):
                nc.scalar.activation(r0[:, mi, :], psr[mi], AF.Sigmoid)
            for mi in range(4):
                nc.scalar.activation(z0[:, mi, :], psz[mi], AF.Sigmoid)
            for mi in range(4):
                nc.scalar.activation(t1[:, mi, :], psn[mi], AF.Identity,
                                     bias=B0NH[:, mi:mi + 1])
            nc.vector.tensor_mul(flat(t1), flat(r0), flat(t1))
            nc.vector.tensor_add(
                flat(t1), flat(t1), XP0[:, 8:12, :].rearrange("p k b -> p (k b)"))
            nc.scalar.activation(flat(n0), flat(t1), AF.Tanh)
            nc.vector.tensor_sub(flat(d0), flat(h0p), flat(n0))
            nc.vector.tensor_mul(flat(d0), flat(z0), flat(d0))
            h0n = sp.tile([128, KT, BC], f32r, tag="h0")
            nc.vector.tensor_add(flat(h0n), flat(n0), flat(d0))
            return h0n

        def emit_l1_gates(psr, psz, psn, psx, h1p):
            r1 = gp.tile([128, KT, BC], f32, tag="r")
            z1 = gp.tile([128, KT, BC], f32, tag="z")
            t1 = gp.tile([128, KT, BC], f32, tag="t1")
            n1 = gp.tile([128, KT, BC], f32, tag="n")
            d1 = gp.tile([128, KT, BC], f32, tag="d")
            for mi in range(4):
                nc.scalar.activation(r1[:, mi, :], psr[mi], AF.Sigmoid,
                                     bias=B1RZ[:, mi:mi + 1])
            for mi in range(4):
                nc.scalar.activation(z1[:, mi, :], psz[mi], AF.Sigmoid,
                                     bias=B1RZ[:, 4 + mi:5 + mi])
            for mi in range(4):
                nc.scalar.activation(t1[:, mi, :], psn[mi], AF.Identity,
                                     bias=B1NH[:, mi:mi + 1])
            nc.vector.tensor_mul(flat(t1), flat(r1), flat(t1))
            nc.vector.tensor_add(flat(t1), flat(t1), flat(psx))
            for mi in range(4):
                nc.scalar.activation(n1[:, mi, :], t1[:, mi, :], AF.Tanh,
                                     bias=B1NX[:, mi:mi + 1])
            nc.vector.tensor_sub(flat(d1), flat(h1p), flat(n1))
            nc.vector.tensor_mul(flat(d1), flat(z1), flat(d1))
            h1n = sp.tile([128, KT, BC], f32r, tag="h1")
            nc.vector.tensor_add(flat(h1n), flat(n1), flat(d1))
            return h1n

        # ---- software-pipelined main loop ----
        # block t: PE burst = L0MM(t) | outMM(t-2) | L1MM(t-1) -- all deps
        # ready at burst start, so the PE never waits mid-block. Gate math
        # for step t (L0) and t-1 (L1) overlaps the next block's burst.
        l1ps = None
        for t in range(T):
            psr0, psz0, psn0 = emit_l0_mm(h0)
            if t >= 2:
                pso = emit_out_mm(h1)
                emit_out_copy(pso, t - 2)
            if t >= 1:
                l1ps = emit_l1_mm(h0, h1)
            h0 = emit_l0_gates(psr0, psz0, psn0, h0)
            if t >= 1:
                h1 = emit_l1_gates(*l1ps, h1)
        # epilogue: finish step T-1's layer 1 and the last two outputs
        if T >= 2:
            pso = emit_out_mm(h1)
            emit_out_copy(pso, T - 2)
        l1ps = emit_l1_mm(h0, h1)
        h1 = emit_l1_gates(*l1ps, h1)
        pso = emit_out_mm(h1)
        emit_out_copy(pso, T - 1)

    nc.compile()
    return nc


class TileCtx:
    """Context manager bundling TileContext + the pools the kernel uses."""

    def __init__(self, nc, tile_mod):
        self.nc = nc
        self.tile = tile_mod
        self._stack = None

    def __enter__(self):
        from contextlib import ExitStack

        self._stack = ExitStack()
        tc = self._stack.enter_context(self.tile.TileContext(self.nc))
        wp = self._stack.enter_context(tc.tile_pool(name="weights", bufs=1))
        cp = self._stack.enter_context(tc.tile_pool(name="consts", bufs=1))
        sp = self._stack.enter_context(tc.tile_pool(name="state", bufs=2))
        gp = self._stack.enter_context(tc.tile_pool(name="work", bufs=2))
        pp = self._stack.enter_context(tc.tile_pool(name="ps", bufs=6, space="PSUM"))
        px = self._stack.enter_context(tc.tile_pool(name="psx", bufs=1, space="PSUM"))
        return tc, (wp, cp, sp, gp, pp, px)

    def __exit__(self, *exc):
        return self._stack.__exit__(*exc)


def _wblk(W):  # W [K, M] -> [128, 4*M] per-partition weight block
    K, M = W.shape
    return W.astype(np.float32).reshape(K // 128, 128, M).transpose(1, 0, 2) \
        .reshape(128, (K // 128) * M)


def _prep_shared(Wih0, Whh0, bih0, bhh0, Wih1, Whh1, bih1, bhh1, Wout, bout):
    """Returns the [128, FTOT] blob with everything but the per-core x part."""
    f = np.float32
    blob = np.zeros((128, FTOT), f)
    blob[:, OFF_W0X:OFF_W0X + 4 * G3] = _wblk(Wih0.T)
    blob[:, OFF_W0H:OFF_W0H + 4 * G3] = _wblk(Whh0.T)
    blob[:, OFF_W1X:OFF_W1X + 4 * G3] = _wblk(Wih1.T)
    blob[:, OFF_W1H:OFF_W1H + 4 * G3] = _wblk(Whh1.T)
    blob[:, OFF_WO:OFF_WO + 4 * OUT] = _wblk(Wout.T)
    blob[:, OFF_EYE:OFF_EYE + 128] = np.eye(128, dtype=f)
    blob[:, OFF_XB0:OFF_XB0 + 12] = np.concatenate(
        [bih0[:2 * H] + bhh0[:2 * H], bih0[2 * H:]]).astype(f).reshape(12, 128).T
    blob[:, OFF_B1RZ:OFF_B1RZ + 8] = \
        (bih1[:2 * H] + bhh1[:2 * H]).astype(f).reshape(8, 128).T
    blob[:, OFF_B0NH:OFF_B0NH + 4] = bhh0[2 * H:].astype(f).reshape(4, 128).T
    blob[:, OFF_B1NH:OFF_B1NH + 4] = bhh1[2 * H:].astype(f).reshape(4, 128).T
    blob[:, OFF_B1NX:OFF_B1NX + 4] = bih1[2 * H:].astype(f).reshape(4, 128).T
    blob[:, OFF_BO:OFF_BO + 2] = bout.astype(f).reshape(2, 128).T
    return blob


def kernel(x, Wih0, Whh0, bih0, bhh0, Wih1, Whh1, bih1, bhh1, Wout, bout,
           pre_set_idx):
    from concourse.bass_utils import run_bass_kernel_spmd

    x = np.asarray(x)
    psi = np.asarray(pre_set_idx).astype(np.int64)
    counts = np.bincount(psi, minlength=B)
    T = int(counts.max())

    if T not in _BUILD_CACHE:
        _BUILD_CACHE[T] = _build(T)
    nc = _BUILD_CACHE[T]

    shared = _prep_shared(np.asarray(Wih0), np.asarray(Whh0), np.asarray(bih0),
                          np.asarray(bhh0), np.asarray(Wih1), np.asarray(Whh1),
                          np.asarray(bih1), np.asarray(bhh1), np.asarray(Wout),
                          np.asarray(bout))
    in_maps = []
    for cid in range(NCORES):
        bl = shared.copy()
        bl[:, OFF_XT:OFF_XT + 4 * BC] = _wblk(
            np.ascontiguousarray(x[cid * BC:(cid + 1) * BC].astype(np.float32).T)
        )
        in_maps.append({"blob": bl})

    trace = os.environ.get("BASS_GRU_TRACE", "") == "1"
    res = run_bass_kernel_spmd(nc, in_maps, core_ids=list(range(NCORES)),
                               trace=trace)
    kernel.last_result = res

    y = np.stack([r["out"] for r in res.results])  # [8, T, 2, 128, BC]

    starts = np.concatenate([[0], np.cumsum(counts)[:-1]])
    rank = np.arange(psi.shape[0]) - starts[psi]
    core = psi // BC
    bl = psi % BC
    sel = y[core, rank, :, :, bl]  # [N, 2, 128]
    return np.ascontiguousarray(sel.reshape(psi.shape[0], OUT))


kernel.last_result = None
